# revision 1
# baseline (speedup 1.0000x reference)
"""Causal self-attention with RoPE on 8 trn2 NeuronCores (~441us HW).

Sharding: tensor-parallel over heads (Megatron style). 16 heads, 8 cores
-> 2 heads per core. Each core computes q/k/v for its 2 heads, causal
attention, and a partial output projection against its w_o column slice.
Host sums the 8 partial outputs (the Megatron all-reduce, done at gather).

Device-side design (bf16 compute, fp32 PSUM accumulation throughout):
 - xT [C, B*T] bf16: x pre-transposed on host so the QKV contraction dim
   (c) is on SBUF partitions; no on-device transpose of x.
 - w_qk packed per head into even/odd-dim column blocks [QE|QO|KE|KO];
   QKV matmuls produce q/k directly in [d, t] layout, head-stacked so
   RoPE runs full-128-partition DVE tensor_tensor ops (6 per tile).
   PSUM is freed via one wide ScalarE copy per tile; rope reads SBUF.
 - v in natural [t, d] layout (same x tiles, shared j/c loop), stored
   with a ones column per head: AV then yields y[tq, 0:128] AND the
   softmax denominator y[:, 128] from a single accumulated matmul.
 - Post-rope q/k repacked (SBUF->SBUF DMA) into per-head [d, t] tensors
   so scores are single K=128 matmuls: scoresT[ts, tq] = khat.T @ qhat.
 - Softmax: no max-subtraction (scores ~N(0,1)); exp on ScalarE with the
   1/sqrt(D) scale folded in, emitted over paired 1024-wide tq blocks to
   amortize per-instruction overhead; causal masking only on
   diagonal-touching tiles via 0/1 bf16 mask multiply.
 - Normalize with reciprocal + per-partition tensor_scalar, PE-transpose
   y -> yT, output projection accumulates both heads in PSUM, ScalarE/DVE
   bounce to SBUF, DMA out as a partial [B*T, C] f32 result.

Compile recipe (this container): bacc.Bacc("TRN2") + TileContext +
nc.finalize() before run_bass_kernel_spmd (bacc legalizes multi-wait
instructions; raw bass.Bass fails walrus codegen here).
"""

import math

import numpy as np

B, T, C, H = 2, 2048, 2048, 16
D = C // H  # 128
NCORES = 8
HPC = H // NCORES  # heads per core = 2
N = B * T  # 4096 token rows
TT = T // 128  # 16 t-tiles per batch
NB = T // 512  # 4 n/tq blocks of 512 per batch
CT = C // 128  # 16 contraction tiles

_COMPILED = None


def _build():
    import concourse.bacc as bacc
    import concourse.mybir as mybir
    import concourse.tile as tile
    from concourse.masks import make_identity

    f32 = mybir.dt.float32
    bf16 = mybir.dt.bfloat16

    nc = bacc.Bacc("TRN2", target_bir_lowering=False, debug=False)
    xT = nc.declare_dram_parameter("xT", [C, N], bf16, isOutput=False)
    w_qk = nc.declare_dram_parameter("w_qk", [C, 4 * D], bf16, isOutput=False)
    w_v = nc.declare_dram_parameter("w_v", [C, HPC * D], bf16, isOutput=False)
    w_o = nc.declare_dram_parameter("w_o", [HPC * D, C], bf16, isOutput=False)
    cos2 = nc.declare_dram_parameter("cos2", [D, N], bf16, isOutput=False)
    sin2 = nc.declare_dram_parameter("sin2", [D, N], bf16, isOutput=False)
    # masks: 4 variants [128,1024] (diag in left half, right half ones)
    # then 4 variants [128,512] (diag within the single block)
    masks = nc.declare_dram_parameter("masks", [128, 4 * 1024 + 4 * 512], bf16, isOutput=False)
    out_p = nc.declare_dram_parameter("out_p", [N, C], f32, isOutput=True)

    SCALE = 1.0 / math.sqrt(D)
    VW = HPC * D + 2 * HPC  # 260: per t-tile v storage [v_h0|1|pad|v_h1|1|pad]

    with tile.TileContext(nc) as tc:
        with (
            tc.tile_pool(name="wpool", bufs=1) as wpool,
            tc.tile_pool(name="xpool", bufs=6) as xpool,
            tc.tile_pool(name="eo", bufs=4) as eopool,
            tc.tile_pool(name="ropetmp", bufs=6) as tmppool,
            tc.tile_pool(name="vsb", bufs=1) as vpool,
            tc.tile_pool(name="expp", bufs=18) as exppool,
            tc.tile_pool(name="ysb", bufs=4) as ypool,
            tc.tile_pool(name="rsb", bufs=8) as rpool,
            tc.tile_pool(name="yts", bufs=3) as ytpool,
            tc.tile_pool(name="pbig", bufs=2, space="PSUM") as pbig,
            tc.tile_pool(name="paux", bufs=4, space="PSUM") as paux,
        ):
            # ---- resident weights / constants ----
            wqk_sb = wpool.tile([128, CT * 512], bf16, tag="wqk")
            nc.sync.dma_start(
                out=wqk_sb[:, :].rearrange("p (kt e) -> p kt e", kt=CT),
                in_=w_qk.rearrange("(kt p) e -> p kt e", p=128),
            )
            wv_sb = wpool.tile([128, CT * 256], bf16, tag="wv")
            nc.sync.dma_start(
                out=wv_sb[:, :].rearrange("p (kt e) -> p kt e", kt=CT),
                in_=w_v.rearrange("(kt p) e -> p kt e", p=128),
            )
            wo_sb = wpool.tile([128, HPC * C], bf16, tag="wo")
            nc.sync.dma_start(
                out=wo_sb[:, :].rearrange("p (kt o) -> p kt o", kt=HPC),
                in_=w_o.rearrange("(kt p) o -> p kt o", p=128),
            )
            cos_sb = wpool.tile([128, N], bf16, tag="cos")
            nc.sync.dma_start(out=cos_sb[:, :], in_=cos2[:, :])
            sin_sb = wpool.tile([128, N], bf16, tag="sin")
            nc.sync.dma_start(out=sin_sb[:, :], in_=sin2[:, :])
            mask_sb = wpool.tile([128, 4 * 1024 + 4 * 512], bf16, tag="mask")
            nc.sync.dma_start(out=mask_sb[:, :], in_=masks[:, :])
            ident = wpool.tile([128, 128], bf16, tag="ident")
            make_identity(nc, ident[:, :])

            v_sb = vpool.tile([128, TT * VW], bf16, tag="vsb")
            for tt in range(TT):
                for h in range(HPC):
                    col = tt * VW + h * 130 + 128
                    nc.vector.memset(v_sb[:, col : col + 1], 1.0)

            for b in range(B):
                n0 = b * T

                # ---- phase QK: q,k projection in [d, t] layout + RoPE ----
                # Two 2-bank psum tiles per j: [QE | QO] and [KE | KO].
                qe2 = eopool.tile([128, T], bf16, tag="eo", name="qe2")
                qo2 = eopool.tile([128, T], bf16, tag="eo", name="qo2")
                ke2 = eopool.tile([128, T], bf16, tag="eo", name="ke2")
                ko2 = eopool.tile([128, T], bf16, tag="eo", name="ko2")
                rot = [(qe2, qo2), (ke2, ko2)]
                qhat = [eopool.tile([128, T], bf16, tag="qh", name=f"qhat{_h}") for _h in range(HPC)]
                khat = [eopool.tile([128, T], bf16, tag="qh", name=f"khat{_h}") for _h in range(HPC)]
                for j in range(NB):
                    js = slice(j * 512, (j + 1) * 512)
                    ps_q = pbig.tile([128, 1024], f32, tag="big", name="ps_q")
                    ps_k = pbig.tile([128, 1024], f32, tag="big", name="ps_k")
                    ps_v = [paux.tile([128, 256], f32, tag="aux", name=f"ps_v{_p}") for _p in range(4)]
                    for c in range(CT):
                        xt = xpool.tile([128, 512], bf16, tag="xt")
                        nc.gpsimd.dma_start(
                            out=xt[:, :],
                            in_=xT[c * 128 : (c + 1) * 128, n0 + j * 512 : n0 + (j + 1) * 512],
                        )
                        for part in range(4):  # QE, QO, KE, KO
                            dst = (ps_q, ps_q, ps_k, ps_k)[part]
                            off = (0, 512, 0, 512)[part]
                            wsl = wqk_sb[:, c * 512 + part * 128 : c * 512 + (part + 1) * 128]
                            nc.tensor.matmul(
                                dst[:, off : off + 512],
                                wsl,
                                xt[:, :],
                                start=(c == 0),
                                stop=(c == CT - 1),
                            )
                        for tl in range(4):
                            nc.tensor.matmul(
                                ps_v[tl][:, :],
                                xt[:, tl * 128 : (tl + 1) * 128],
                                wv_sb[:, c * 256 : (c + 1) * 256],
                                start=(c == 0),
                                stop=(c == CT - 1),
                            )
                    ce = cos_sb[:, n0 + j * 512 : n0 + (j + 1) * 512]
                    se = sin_sb[:, n0 + j * 512 : n0 + (j + 1) * 512]
                    # One wide ACT copy per psum tile frees the banks fast;
                    # rope then runs from SBUF off the PE critical path.
                    for qk in range(2):  # 0 = q, 1 = k
                        pc = tmppool.tile([128, 1024], f32, tag="rt", name=f"pc{qk}")
                        nc.scalar.copy(pc[:, :], (ps_q, ps_k)[qk][:, :])
                        E_sb, O_sb = pc[:, 0:512], pc[:, 512:1024]
                        dst_e, dst_o = rot[qk]
                        t1 = tmppool.tile([128, 512], f32, tag="rt2")
                        t2 = tmppool.tile([128, 512], f32, tag="rt2")
                        nc.vector.tensor_mul(t1[:, :], E_sb, ce)
                        nc.vector.tensor_mul(t2[:, :], O_sb, se)
                        nc.vector.tensor_sub(dst_e[:, js], t1[:, :], t2[:, :])
                        t3 = tmppool.tile([128, 512], f32, tag="rt2")
                        t4 = tmppool.tile([128, 512], f32, tag="rt2")
                        nc.vector.tensor_mul(t3[:, :], E_sb, se)
                        nc.vector.tensor_mul(t4[:, :], O_sb, ce)
                        nc.vector.tensor_add(dst_o[:, js], t3[:, :], t4[:, :])
                    for tl in range(4):
                        tt = j * 4 + tl
                        base = tt * VW
                        for h in range(HPC):
                            nc.vector.tensor_copy(
                                v_sb[:, base + h * 130 : base + h * 130 + 128],
                                ps_v[tl][:, h * 128 : (h + 1) * 128],
                            )
                    for h in range(HPC):
                        hb = 64 * h
                        nc.sync.dma_start(out=qhat[h][0:64, js], in_=qe2[hb : hb + 64, js])
                        nc.sync.dma_start(out=qhat[h][64:128, js], in_=qo2[hb : hb + 64, js])
                        nc.sync.dma_start(out=khat[h][0:64, js], in_=ke2[hb : hb + 64, js])
                        nc.sync.dma_start(out=khat[h][64:128, js], in_=ko2[hb : hb + 64, js])

                # ---- attention per head: paired tq blocks (jlo, jhi) share
                # one [128,1024] score psum + one wide exp instruction ----
                yT = [eopool.tile([128, T], bf16, tag="yt", name=f"yT{_h}") for _h in range(HPC)]
                for h in range(HPC):
                    for jp in range(NB // 2):
                        jlo, jhi = 2 * jp, 2 * jp + 1
                        exp_of = {}  # i -> (tile, base col of jlo half or None)
                        for i in range(4 * jhi + 4):
                            isl = slice(i * 128, (i + 1) * 128)
                            combined = i <= 4 * jlo + 3
                            sc = pbig.tile([128, 1024], f32, tag="big", name="sc")
                            ex = exppool.tile([128, 1024], bf16, tag="ex")
                            if combined:
                                nc.tensor.matmul(
                                    sc[:, 0:512], khat[h][:, isl],
                                    qhat[h][:, jlo * 512 : (jlo + 1) * 512],
                                    start=True, stop=True,
                                )
                                nc.tensor.matmul(
                                    sc[:, 512:1024], khat[h][:, isl],
                                    qhat[h][:, jhi * 512 : (jhi + 1) * 512],
                                    start=True, stop=True,
                                )
                                nc.scalar.activation(
                                    ex[:, :], sc[:, :],
                                    mybir.ActivationFunctionType.Exp, scale=SCALE,
                                )
                                p = i - 4 * jlo
                                if p >= 0:
                                    nc.vector.tensor_mul(
                                        ex[:, :], ex[:, :],
                                        mask_sb[:, p * 1024 : (p + 1) * 1024],
                                    )
                                exp_of[i] = (ex, 0)
                            else:
                                nc.tensor.matmul(
                                    sc[:, 0:512], khat[h][:, isl],
                                    qhat[h][:, jhi * 512 : (jhi + 1) * 512],
                                    start=True, stop=True,
                                )
                                nc.scalar.activation(
                                    ex[:, 0:512], sc[:, 0:512],
                                    mybir.ActivationFunctionType.Exp, scale=SCALE,
                                )
                                p = i - 4 * jhi
                                if p >= 0:
                                    nc.vector.tensor_mul(
                                        ex[:, 0:512], ex[:, 0:512],
                                        mask_sb[:, 4096 + p * 512 : 4096 + (p + 1) * 512],
                                    )
                                exp_of[i] = (ex, None)

                        for j in (jlo, jhi):
                            half = 0 if j == jlo else 512
                            y_ps = [paux.tile([128, 129], f32, tag="aux", name=f"y_ps{_p}") for _p in range(4)]
                            for tau in range(4):
                                g = 4 * j + tau
                                for i in range(g + 1):
                                    ex, base = exp_of[i]
                                    col = (half if base == 0 else 0) + tau * 128
                                    nc.tensor.matmul(
                                        y_ps[tau][:, :],
                                        ex[:, col : col + 128],
                                        v_sb[:, i * VW + h * 130 : i * VW + h * 130 + 129],
                                        start=(i == 0),
                                        stop=(i == g),
                                    )
                            for tau in range(4):
                                g = 4 * j + tau
                                r = rpool.tile([128, 1], f32, tag="r")
                                nc.vector.reciprocal(r[:, :], y_ps[tau][:, 128:129])
                                y_sb = ypool.tile([128, 128], bf16, tag="y")
                                nc.vector.tensor_scalar_mul(
                                    y_sb[:, :], y_ps[tau][:, 0:128], r[:, 0:1]
                                )
                                yt_ps = paux.tile([128, 128], bf16, tag="aux")
                                nc.tensor.transpose(yt_ps[:, :], y_sb[:, :], ident[:, :])
                                nc.vector.tensor_copy(
                                    yT[h][:, g * 128 : (g + 1) * 128], yt_ps[:, :]
                                )

                # ---- output projection (partial over this core's heads) ----
                for tt in range(TT):
                    tsl = slice(tt * 128, (tt + 1) * 128)
                    for obp in range(2):  # pairs of 512-wide o blocks
                        o_ps = pbig.tile([128, 1024], f32, tag="big", name="o_ps")
                        for ob in (2 * obp, 2 * obp + 1):
                            off = (ob - 2 * obp) * 512
                            for h in range(HPC):
                                nc.tensor.matmul(
                                    o_ps[:, off : off + 512],
                                    yT[h][:, tsl],
                                    wo_sb[:, h * C + ob * 512 : h * C + (ob + 1) * 512],
                                    start=(h == 0),
                                    stop=(h == HPC - 1),
                                )
                        yo = ytpool.tile([128, 1024], f32, tag="yo")
                        nc.vector.tensor_copy(yo[:, :], o_ps[:, :])
                        nc.sync.dma_start(
                            out=out_p[n0 + tt * 128 : n0 + (tt + 1) * 128, obp * 1024 : (obp + 1) * 1024],
                            in_=yo[:, :],
                        )
    nc.finalize()
    return nc


def _prep_inputs(x, w_qkv, w_o, rope_cos, rope_sin):
    import ml_dtypes

    bf = ml_dtypes.bfloat16
    xTh = np.ascontiguousarray(x.reshape(N, C).T).astype(bf)
    cosT = np.ascontiguousarray(rope_cos.T)  # [64, T]
    sinT = np.ascontiguousarray(rope_sin.T)
    cos2 = np.tile(np.concatenate([cosT, cosT], 0), (1, B)).astype(bf)
    sin2 = np.tile(np.concatenate([sinT, sinT], 0), (1, B)).astype(bf)

    r = np.arange(128)[:, None]
    c = np.arange(512)[None, :]
    singles = [((c - r) >= 128 * p).astype(np.float32) for p in range(4)]
    ones512 = np.ones((128, 512), dtype=np.float32)
    combos = [np.concatenate([s, ones512], 1) for s in singles]
    mk = np.concatenate(combos + singles, axis=1).astype(bf)

    ev = np.arange(0, D, 2)
    od = np.arange(1, D, 2)
    in_maps = []
    for m in range(NCORES):
        h0, h1 = 2 * m, 2 * m + 1
        # blocks QE|QO|KE|KO; within each, cols = [head0 dims | head1 dims]
        QE = np.concatenate([w_qkv[h0 * D + ev, :], w_qkv[h1 * D + ev, :]], 0).T
        QO = np.concatenate([w_qkv[h0 * D + od, :], w_qkv[h1 * D + od, :]], 0).T
        KE = np.concatenate([w_qkv[C + h0 * D + ev, :], w_qkv[C + h1 * D + ev, :]], 0).T
        KO = np.concatenate([w_qkv[C + h0 * D + od, :], w_qkv[C + h1 * D + od, :]], 0).T
        wqk_m = np.ascontiguousarray(np.concatenate([QE, QO, KE, KO], 1)).astype(bf)
        wv_m = np.ascontiguousarray(
            w_qkv[2 * C + 2 * m * D : 2 * C + (2 * m + 2) * D, :].T
        ).astype(bf)
        wo_m = np.ascontiguousarray(w_o[:, 2 * m * D : (2 * m + 2) * D].T).astype(bf)
        in_maps.append(
            {
                "xT": xTh,
                "w_qk": wqk_m,
                "w_v": wv_m,
                "w_o": wo_m,
                "cos2": cos2,
                "sin2": sin2,
                "masks": np.ascontiguousarray(mk),
            }
        )
    return in_maps


def kernel(x, w_qkv, w_o, rope_cos, rope_sin, _trace=False):
    global _COMPILED
    x = np.asarray(x, dtype=np.float32)
    w_qkv = np.asarray(w_qkv, dtype=np.float32)
    w_o = np.asarray(w_o, dtype=np.float32)
    rope_cos = np.asarray(rope_cos, dtype=np.float32)
    rope_sin = np.asarray(rope_sin, dtype=np.float32)

    from concourse.bass_utils import run_bass_kernel_spmd

    if _COMPILED is None:
        _COMPILED = _build()
    nc = _COMPILED
    in_maps = _prep_inputs(x, w_qkv, w_o, rope_cos, rope_sin)
    res = run_bass_kernel_spmd(
        nc, in_maps, core_ids=list(range(NCORES)), trace=_trace
    )
    out = np.zeros((N, C), dtype=np.float32)
    for m in range(NCORES):
        out += res.results[m]["out_p"]
    kernel._last_results = res
    return out.reshape(B, T, C)



# revision 2
# speedup vs baseline: 1.2343x; 1.2343x over previous
"""Causal self-attention with RoPE on 8 trn2 NeuronCores.

Sharding: tensor-parallel over heads (Megatron style). 16 heads, 8 cores
-> 2 heads per core. Each core computes q/k/v for its 2 heads, causal
attention, and a partial output projection against its w_o column slice.
Host sums the 8 partial outputs (the Megatron all-reduce, done at gather).

v2: software-pipelined emission. The TRN2 PE clock p-states (2.4 GHz only
after 3us of continuous execution, 1.2 GHz after any idle gap) make PE
gaps extremely expensive, so the kernel is emitted as one interleaved
stream: attention of 512-token block j (ACT-exp / DVE-heavy) is
interleaved at instruction granularity with the QKV projection of block
j+1 and the output projection of block j-1 (both pure PE) as "filler".

Per 512-token block j (8 blocks = 2 batches x 4):
 - Q-pass / K-pass / V-pass: three passes over the SBUF-resident x tiles
   of the block, each accumulating in a small PSUM footprint (2 banks qk,
   1 bank v) so attention + WO psum fits alongside: qk 2 + v/wo 2 +
   scores 3 + av 1 = 8 banks.
 - RoPE applied from a bf16 staging copy of the q/k PSUM, writing
   qhat/khat (per-head [d, t] layout) directly with partition-sliced DVE
   ops (no repack DMAs).
 - Attention per head: per 128-key-tile i: scoresT[ts,tq] single matmul
   (khat_i stationary, qhat_j moving), exp on ACT (scale folded),
   causal 0/1 mask multiply on diagonal tiles; then per 128-query tile:
   AV chain over v tiles with a ones column producing y and the softmax
   denominator in one accumulation; normalize, PE-transpose to yT.
 - WO: per 128-token tile, 4x 512-wide chains over both heads, drained
   alternately on ACT/DVE, DMA'd out as bf16 partials (summed on host).
"""

import math

import numpy as np

B, T, C, H = 2, 2048, 2048, 16
D = C // H  # 128
NCORES = 8
HPC = H // NCORES  # heads per core = 2
N = B * T  # 4096 token rows
NB = T // 512  # 4 blocks of 512 per batch
NBLK = B * NB  # 8 global 512-token blocks
CT = C // 128  # 16 contraction tiles
VW = HPC * D + 2 * HPC  # 260: per t-tile v storage [v_h0|1|pad|v_h1|1|pad]

_COMPILED = None


def _build():
    import concourse.bacc as bacc
    import concourse.mybir as mybir
    import concourse.tile as tile
    from concourse.masks import make_identity

    f32 = mybir.dt.float32
    bf16 = mybir.dt.bfloat16

    nc = bacc.Bacc("TRN2", target_bir_lowering=False, debug=False)
    xT = nc.declare_dram_parameter("xT", [C, N], bf16, isOutput=False)
    w_qk = nc.declare_dram_parameter("w_qk", [C, 4 * D], bf16, isOutput=False)
    w_v = nc.declare_dram_parameter("w_v", [C, HPC * D], bf16, isOutput=False)
    w_o = nc.declare_dram_parameter("w_o", [HPC * D, C], bf16, isOutput=False)
    cos2 = nc.declare_dram_parameter("cos2", [D, N], bf16, isOutput=False)
    sin2 = nc.declare_dram_parameter("sin2", [D, N], bf16, isOutput=False)
    masks = nc.declare_dram_parameter("masks", [128, 4 * 512], bf16, isOutput=False)
    out_p = nc.declare_dram_parameter("out_p", [N, C], bf16, isOutput=True)

    SCALE = 1.0 / math.sqrt(D)

    with tile.TileContext(nc) as tc:
        with (
            tc.tile_pool(name="wpool", bufs=1) as wpool,
            tc.tile_pool(name="xpool", bufs=1) as xpool,
            tc.tile_pool(name="qkh", bufs=1) as qkhpool,
            tc.tile_pool(name="pcp", bufs=2) as pcpool,
            tc.tile_pool(name="rtmp", bufs=4) as rtpool,
            tc.tile_pool(name="expp", bufs=20) as expool,
            tc.tile_pool(name="ysb", bufs=3) as ypool,
            tc.tile_pool(name="rsb", bufs=3) as rpool,
            tc.tile_pool(name="yop", bufs=2) as yopool,
            tc.tile_pool(name="pqk", bufs=1, space="PSUM") as pqk,
            tc.tile_pool(name="pf", bufs=2, space="PSUM") as pf,
            tc.tile_pool(name="psc", bufs=3, space="PSUM") as psc,
            tc.tile_pool(name="py", bufs=1, space="PSUM") as py,
        ):
            # ---- resident weights / constants ----
            wqk_sb = wpool.tile([128, CT * 512], bf16, tag="wqk")
            nc.sync.dma_start(
                out=wqk_sb[:, :].rearrange("p (kt e) -> p kt e", kt=CT),
                in_=w_qk.rearrange("(kt p) e -> p kt e", p=128),
            )
            wv_sb = wpool.tile([128, CT * 256], bf16, tag="wv")
            nc.sync.dma_start(
                out=wv_sb[:, :].rearrange("p (kt e) -> p kt e", kt=CT),
                in_=w_v.rearrange("(kt p) e -> p kt e", p=128),
            )
            wo_sb = wpool.tile([128, HPC * C], bf16, tag="wo")
            nc.sync.dma_start(
                out=wo_sb[:, :].rearrange("p (kt o) -> p kt o", kt=HPC),
                in_=w_o.rearrange("(kt p) o -> p kt o", p=128),
            )
            cos_sb = wpool.tile([128, N], bf16, tag="cos")
            nc.sync.dma_start(out=cos_sb[:, :], in_=cos2[:, :])
            sin_sb = wpool.tile([128, N], bf16, tag="sin")
            nc.sync.dma_start(out=sin_sb[:, :], in_=sin2[:, :])
            mask_sb = wpool.tile([128, 4 * 512], bf16, tag="mask")
            nc.sync.dma_start(out=mask_sb[:, :], in_=masks[:, :])
            ident = wpool.tile([128, 128], bf16, tag="ident")
            make_identity(nc, ident[:, :])

            # persistent per-batch state
            v_sb = [wpool.tile([128, 4 * NB * VW], bf16, tag=f"vsb{b}", name=f"v_sb{b}") for b in range(B)]
            for b in range(B):
                for tt in range(4 * NB):
                    for h in range(HPC):
                        col = tt * VW + h * 130 + 128
                        nc.vector.memset(v_sb[b][:, col : col + 1], 1.0)
            khat = [
                [wpool.tile([128, T], bf16, tag=f"kh{b}{h}", name=f"khat{b}{h}") for h in range(HPC)]
                for b in range(B)
            ]
            yT = [
                [wpool.tile([128, T], bf16, tag=f"yt{b}{h}", name=f"yT{b}{h}") for h in range(HPC)]
                for b in range(B)
            ]
            # qhat double-buffered by block parity
            qhat = [
                [qkhpool.tile([128, 512], bf16, tag=f"qh{p}{h}", name=f"qhat{p}{h}") for h in range(HPC)]
                for p in range(2)
            ]
            # x tiles double-buffered by block parity: 16 tiles of [128,512]
            xts = [
                [xpool.tile([128, 512], bf16, tag=f"x{p}_{c}", name=f"xt{p}_{c}") for c in range(CT)]
                for p in range(2)
            ]

            def prefetch_x(gj):
                p = gj % 2
                for c in range(CT):
                    nc.gpsimd.dma_start(
                        out=xts[p][c][:, :],
                        in_=xT[c * 128 : (c + 1) * 128, gj * 512 : (gj + 1) * 512],
                    )

            def rope_drain(ps, gj, dst_of_h):
                """ps = [E(512)|O(512)] psum f32; write rotated per-head
                [d,512] into dst_of_h[h] (cols 0:512 of qhat, or the j-block
                cols of khat)."""
                pc = pcpool.tile([128, 1024], bf16, tag="pc")
                nc.scalar.copy(pc[:, :], ps[:, :])
                E, O = pc[:, 0:512], pc[:, 512:1024]
                ce = cos_sb[:, gj * 512 : (gj + 1) * 512]
                se = sin_sb[:, gj * 512 : (gj + 1) * 512]
                t1 = rtpool.tile([128, 512], bf16, tag="rt")
                t2 = rtpool.tile([128, 512], bf16, tag="rt")
                nc.vector.tensor_mul(t1[:, :], E, ce)
                nc.vector.tensor_mul(t2[:, :], O, se)
                for h in range(HPC):
                    hb = 64 * h
                    dst, c0 = dst_of_h[h]
                    nc.vector.tensor_sub(
                        dst[0:64, c0 : c0 + 512], t1[hb : hb + 64, :], t2[hb : hb + 64, :]
                    )
                t3 = rtpool.tile([128, 512], bf16, tag="rt")
                t4 = rtpool.tile([128, 512], bf16, tag="rt")
                nc.vector.tensor_mul(t3[:, :], E, se)
                nc.vector.tensor_mul(t4[:, :], O, ce)
                for h in range(HPC):
                    hb = 64 * h
                    dst, c0 = dst_of_h[h]
                    nc.vector.tensor_add(
                        dst[64:128, c0 : c0 + 512], t3[hb : hb + 64, :], t4[hb : hb + 64, :]
                    )

            def qkv_units(gj):
                """PE filler units for block gj's projections."""
                b, j = divmod(gj, NB)
                p = gj % 2
                units = []

                def qk_pass(part0, drain):
                    # part0 = 0 for Q (QE,QO), 2 for K (KE,KO)
                    ps = pqk.tile([128, 1024], f32, tag="qk", name="ps_qk")

                    def unit(c0, ps=ps):
                        for c in range(c0, c0 + 2):
                            for e in range(2):
                                pt = part0 + e
                                wsl = wqk_sb[:, c * 512 + pt * 128 : c * 512 + (pt + 1) * 128]
                                nc.tensor.matmul(
                                    ps[:, e * 512 : (e + 1) * 512],
                                    wsl,
                                    xts[p][c][:, :],
                                    start=(c == 0),
                                    stop=(c == CT - 1),
                                )
                        if c0 + 2 == CT:
                            drain(ps)
                    return [lambda c0=c0: unit(c0) for c0 in range(0, CT, 2)]

                units += qk_pass(
                    0, lambda ps: rope_drain(ps, gj, [(qhat[p][h], 0) for h in range(HPC)])
                )
                units += qk_pass(
                    2, lambda ps: rope_drain(ps, gj, [(khat[b][h], j * 512) for h in range(HPC)])
                )

                def v_unit(tl):
                    ps = pf.tile([128, 256], f32, tag="f", name="ps_v")
                    for c in range(CT):
                        nc.tensor.matmul(
                            ps[:, :],
                            xts[p][c][:, tl * 128 : (tl + 1) * 128],
                            wv_sb[:, c * 256 : (c + 1) * 256],
                            start=(c == 0),
                            stop=(c == CT - 1),
                        )
                    base = (j * 4 + tl) * VW
                    for h in range(HPC):
                        nc.scalar.copy(
                            v_sb[b][:, base + h * 130 : base + h * 130 + 128],
                            ps[:, h * 128 : (h + 1) * 128],
                        )
                units += [lambda tl=tl: v_unit(tl) for tl in range(4)]
                return units

            def wo_units(gj):
                """PE filler units for block gj's output projection."""
                b, j = divmod(gj, NB)
                units = []

                def tt_unit(tt, ob2):
                    # two 512-wide o-chains for token tile tt
                    if ob2 == 0:
                        yo = yopool.tile([128, 2048], bf16, tag="yo", name=f"yo_{gj}_{tt}")
                        tt_unit.yo = yo
                    yo = tt_unit.yo
                    tsl = slice((j * 4 + tt) * 128, (j * 4 + tt) * 128 + 128)
                    for ob in (2 * ob2, 2 * ob2 + 1):
                        ps = pf.tile([128, 512], f32, tag="f", name="o_ps")
                        for h in range(HPC):
                            nc.tensor.matmul(
                                ps[:, :],
                                yT[b][h][:, tsl],
                                wo_sb[:, h * C + ob * 512 : h * C + (ob + 1) * 512],
                                start=(h == 0),
                                stop=(h == HPC - 1),
                            )
                        dsl = yo[:, ob * 512 : (ob + 1) * 512]
                        if ob % 2 == 0:
                            nc.scalar.copy(dsl, ps[:, :])
                        else:
                            nc.vector.tensor_copy(dsl, ps[:, :])
                    if ob2 == 1:
                        n0 = b * T + (j * 4 + tt) * 128
                        nc.sync.dma_start(out=out_p[n0 : n0 + 128, :], in_=yo[:, :])
                for tt in range(4):
                    for ob2 in range(2):
                        units.append(lambda tt=tt, ob2=ob2: tt_unit(tt, ob2))
                return units

            def attention(gj, filler):
                """Emit attention for block gj, pulling filler units between
                dependency-stalled PE instructions."""
                b, j = divmod(gj, NB)
                p = gj % 2

                def pull(k=1):
                    for _ in range(k):
                        if filler:
                            filler.popleft()()

                for h in range(HPC):
                    exs = []
                    for i in range(4 * j + 4):
                        sc = psc.tile([128, 512], f32, tag="sc", name="sc")
                        nc.tensor.matmul(
                            sc[:, :],
                            khat[b][h][:, i * 128 : (i + 1) * 128],
                            qhat[p][h][:, :],
                            start=True,
                            stop=True,
                        )
                        ex = expool.tile([128, 512], bf16, tag="ex")
                        nc.scalar.activation(
                            ex[:, :], sc[:, :],
                            mybir.ActivationFunctionType.Exp, scale=SCALE,
                        )
                        pdiag = i - 4 * j
                        if pdiag >= 0:
                            nc.vector.tensor_mul(
                                ex[:, :], ex[:, :],
                                mask_sb[:, pdiag * 512 : (pdiag + 1) * 512],
                            )
                        exs.append(ex)
                        pull()
                    for tau in range(4):
                        g = 4 * j + tau
                        y_ps = py.tile([128, 129], f32, tag="y", name="y_ps")
                        for i in range(g + 1):
                            nc.tensor.matmul(
                                y_ps[:, :],
                                exs[i][:, tau * 128 : (tau + 1) * 128],
                                v_sb[b][:, i * VW + h * 130 : i * VW + h * 130 + 129],
                                start=(i == 0),
                                stop=(i == g),
                            )
                        r = rpool.tile([128, 1], f32, tag="r")
                        nc.vector.reciprocal(r[:, :], y_ps[:, 128:129])
                        y_sb = ypool.tile([128, 128], bf16, tag="y")
                        nc.vector.tensor_scalar_mul(y_sb[:, :], y_ps[:, 0:128], r[:, 0:1])
                        yt_ps = psc.tile([128, 128], bf16, tag="sc", name="yt_ps")
                        nc.tensor.transpose(yt_ps[:, :], y_sb[:, :], ident[:, :])
                        gcol = (j * 4 + tau) * 128
                        nc.vector.tensor_copy(yT[b][h][:, gcol : gcol + 128], yt_ps[:, :])
                        pull()

            # ---------------- schedule ----------------
            from collections import deque

            prefetch_x(0)
            prefetch_x(1)
            for u in qkv_units(0):
                u()
            backlog = deque()
            for gj in range(NBLK):
                if gj + 2 < NBLK:
                    prefetch_x(gj + 2)
                filler = deque()
                if gj + 1 < NBLK:
                    filler.extend(qkv_units(gj + 1))
                filler.extend(backlog)
                backlog.clear()
                attention(gj, filler)
                while filler:
                    filler.popleft()()
                backlog.extend(wo_units(gj))
            for u in backlog:
                u()
    nc.finalize()
    return nc


def _prep_inputs(x, w_qkv, w_o, rope_cos, rope_sin):
    import ml_dtypes

    bf = ml_dtypes.bfloat16
    xTh = np.ascontiguousarray(x.reshape(N, C).T).astype(bf)
    cosT = np.ascontiguousarray(rope_cos.T)  # [64, T]
    sinT = np.ascontiguousarray(rope_sin.T)
    cos2 = np.tile(np.concatenate([cosT, cosT], 0), (1, B)).astype(bf)
    sin2 = np.tile(np.concatenate([sinT, sinT], 0), (1, B)).astype(bf)

    r = np.arange(128)[:, None]
    c = np.arange(512)[None, :]
    singles = [((c - r) >= 128 * p).astype(np.float32) for p in range(4)]
    mk = np.concatenate(singles, axis=1).astype(bf)

    ev = np.arange(0, D, 2)
    od = np.arange(1, D, 2)
    in_maps = []
    for m in range(NCORES):
        h0, h1 = 2 * m, 2 * m + 1
        # blocks QE|QO|KE|KO; within each, cols = [head0 dims | head1 dims]
        QE = np.concatenate([w_qkv[h0 * D + ev, :], w_qkv[h1 * D + ev, :]], 0).T
        QO = np.concatenate([w_qkv[h0 * D + od, :], w_qkv[h1 * D + od, :]], 0).T
        KE = np.concatenate([w_qkv[C + h0 * D + ev, :], w_qkv[C + h1 * D + ev, :]], 0).T
        KO = np.concatenate([w_qkv[C + h0 * D + od, :], w_qkv[C + h1 * D + od, :]], 0).T
        wqk_m = np.ascontiguousarray(np.concatenate([QE, QO, KE, KO], 1)).astype(bf)
        wv_m = np.ascontiguousarray(
            w_qkv[2 * C + 2 * m * D : 2 * C + (2 * m + 2) * D, :].T
        ).astype(bf)
        wo_m = np.ascontiguousarray(w_o[:, 2 * m * D : (2 * m + 2) * D].T).astype(bf)
        in_maps.append(
            {
                "xT": xTh,
                "w_qk": wqk_m,
                "w_v": wv_m,
                "w_o": wo_m,
                "cos2": cos2,
                "sin2": sin2,
                "masks": np.ascontiguousarray(mk),
            }
        )
    return in_maps


def kernel(x, w_qkv, w_o, rope_cos, rope_sin, _trace=False):
    global _COMPILED
    x = np.asarray(x, dtype=np.float32)
    w_qkv = np.asarray(w_qkv, dtype=np.float32)
    w_o = np.asarray(w_o, dtype=np.float32)
    rope_cos = np.asarray(rope_cos, dtype=np.float32)
    rope_sin = np.asarray(rope_sin, dtype=np.float32)

    from concourse.bass_utils import run_bass_kernel_spmd

    if _COMPILED is None:
        _COMPILED = _build()
    nc = _COMPILED
    in_maps = _prep_inputs(x, w_qkv, w_o, rope_cos, rope_sin)
    res = run_bass_kernel_spmd(
        nc, in_maps, core_ids=list(range(NCORES)), trace=_trace
    )
    out = np.zeros((N, C), dtype=np.float32)
    for m in range(NCORES):
        out += np.asarray(res.results[m]["out_p"], dtype=np.float32)
    kernel._last_results = res
    return out.reshape(B, T, C)


# revision 6
# speedup vs baseline: 1.2738x; 1.0320x over previous
"""Causal self-attention with RoPE on 8 trn2 NeuronCores.

Sharding: tensor-parallel over heads (Megatron style). 16 heads, 8 cores
-> 2 heads per core. Each core computes q/k/v for its 2 heads, causal
attention, and a partial output projection against its w_o column slice.
Host sums the 8 partial outputs (the Megatron all-reduce, done at gather).

v2: software-pipelined emission. The TRN2 PE clock p-states (2.4 GHz only
after 3us of continuous execution, 1.2 GHz after any idle gap) make PE
gaps extremely expensive, so the kernel is emitted as one interleaved
stream: attention of 512-token block j (ACT-exp / DVE-heavy) is
interleaved at instruction granularity with the QKV projection of block
j+1 and the output projection of block j-1 (both pure PE) as "filler".

Per 512-token block j (8 blocks = 2 batches x 4):
 - Q-pass / K-pass / V-pass: three passes over the SBUF-resident x tiles
   of the block, each accumulating in a small PSUM footprint (2 banks qk,
   1 bank v) so attention + WO psum fits alongside: qk 2 + v/wo 2 +
   scores 3 + av 1 = 8 banks.
 - RoPE applied from a bf16 staging copy of the q/k PSUM, writing
   qhat/khat (per-head [d, t] layout) directly with partition-sliced DVE
   ops (no repack DMAs).
 - Attention per head: per 128-key-tile i: scoresT[ts,tq] single matmul
   (khat_i stationary, qhat_j moving), exp on ACT (scale folded),
   causal 0/1 mask multiply on diagonal tiles; then per 128-query tile:
   AV chain over v tiles with a ones column producing y and the softmax
   denominator in one accumulation; normalize, PE-transpose to yT.
 - WO: per 128-token tile, 4x 512-wide chains over both heads, drained
   alternately on ACT/DVE, DMA'd out as bf16 partials (summed on host).
"""

import math

import numpy as np

B, T, C, H = 2, 2048, 2048, 16
D = C // H  # 128
NCORES = 8
HPC = H // NCORES  # heads per core = 2
N = B * T  # 4096 token rows
NB = T // 512  # 4 blocks of 512 per batch
NBLK = B * NB  # 8 global 512-token blocks
CT = C // 128  # 16 contraction tiles
VW = HPC * D + 2 * HPC  # 260: per t-tile v storage [v_h0|1|pad|v_h1|1|pad]

_COMPILED = None


def _build():
    import concourse.bacc as bacc
    import concourse.mybir as mybir
    import concourse.tile as tile
    from concourse.masks import make_identity

    f32 = mybir.dt.float32
    bf16 = mybir.dt.bfloat16

    nc = bacc.Bacc("TRN2", target_bir_lowering=False, debug=False)
    xT = nc.declare_dram_parameter("xT", [C, N], bf16, isOutput=False)
    w_qk = nc.declare_dram_parameter("w_qk", [C, 4 * D], bf16, isOutput=False)
    w_v = nc.declare_dram_parameter("w_v", [C, HPC * D], bf16, isOutput=False)
    w_o = nc.declare_dram_parameter("w_o", [HPC * D, C], bf16, isOutput=False)
    cos2 = nc.declare_dram_parameter("cos2", [D, N], bf16, isOutput=False)
    sin2 = nc.declare_dram_parameter("sin2", [D, N], bf16, isOutput=False)
    masks = nc.declare_dram_parameter("masks", [128, 4 * 512], bf16, isOutput=False)
    out_p = nc.declare_dram_parameter("out_p", [N, C], bf16, isOutput=True)

    SCALE = 1.0 / math.sqrt(D)

    with tile.TileContext(nc) as tc:
        with (
            tc.tile_pool(name="wpool", bufs=1) as wpool,
            tc.tile_pool(name="xpool", bufs=1) as xpool,
            tc.tile_pool(name="qkh", bufs=1) as qkhpool,
            tc.tile_pool(name="pcp", bufs=2) as pcpool,
            tc.tile_pool(name="rtmp", bufs=4) as rtpool,
            tc.tile_pool(name="expp", bufs=20) as expool,
            tc.tile_pool(name="ysb", bufs=3) as ypool,
            tc.tile_pool(name="rsb", bufs=3) as rpool,
            tc.tile_pool(name="yop", bufs=2) as yopool,
            tc.tile_pool(name="pf", bufs=2, space="PSUM") as pf,
            tc.tile_pool(name="psc", bufs=4, space="PSUM") as psc,
            tc.tile_pool(name="py", bufs=2, space="PSUM") as py,
        ):
            # ---- resident weights / constants ----
            # wqk first (first consumer), chunked so the first Q-pass matmul
            # only waits on the first quarter.
            wqk_sb = wpool.tile([128, CT * 512], bf16, tag="wqk")
            for cc in range(0, CT, 4):
                nc.sync.dma_start(
                    out=wqk_sb[:, cc * 512 : (cc + 4) * 512].rearrange(
                        "p (kt e) -> p kt e", kt=4
                    ),
                    in_=w_qk.rearrange("(kt p) e -> p kt e", p=128)[
                        :, cc : cc + 4, :
                    ],
                )
            wv_sb = wpool.tile([128, CT * 256], bf16, tag="wv")
            nc.sync.dma_start(
                out=wv_sb[:, :].rearrange("p (kt e) -> p kt e", kt=CT),
                in_=w_v.rearrange("(kt p) e -> p kt e", p=128),
            )
            cos_sb = wpool.tile([128, N], bf16, tag="cos")
            nc.sync.dma_start(out=cos_sb[:, :], in_=cos2[:, :])
            sin_sb = wpool.tile([128, N], bf16, tag="sin")
            nc.sync.dma_start(out=sin_sb[:, :], in_=sin2[:, :])
            mask_sb = wpool.tile([128, 4 * 512], bf16, tag="mask")
            nc.sync.dma_start(out=mask_sb[:, :], in_=masks[:, :])
            wo_sb = wpool.tile([128, HPC * C], bf16, tag="wo")
            nc.sync.dma_start(
                out=wo_sb[:, :].rearrange("p (kt o) -> p kt o", kt=HPC),
                in_=w_o.rearrange("(kt p) o -> p kt o", p=128),
            )
            ident = wpool.tile([128, 128], bf16, tag="ident")
            make_identity(nc, ident[:, :])

            # persistent per-batch state
            v_sb = [wpool.tile([128, 4 * NB * VW], bf16, tag=f"vsb{b}", name=f"v_sb{b}") for b in range(B)]
            for b in range(B):
                for tt in range(4 * NB):
                    for h in range(HPC):
                        col = tt * VW + h * 130 + 128
                        nc.vector.memset(v_sb[b][:, col : col + 1], 1.0)
            khat = [
                [wpool.tile([128, T], bf16, tag=f"kh{b}{h}", name=f"khat{b}{h}") for h in range(HPC)]
                for b in range(B)
            ]
            yT = [
                [wpool.tile([128, T], bf16, tag=f"yt{b}{h}", name=f"yT{b}{h}") for h in range(HPC)]
                for b in range(B)
            ]
            # qhat double-buffered by block parity
            qhat = [
                [qkhpool.tile([128, 512], bf16, tag=f"qh{p}{h}", name=f"qhat{p}{h}") for h in range(HPC)]
                for p in range(2)
            ]
            # x tiles double-buffered by block parity: 16 tiles of [128,512]
            xts = [
                [xpool.tile([128, 512], bf16, tag=f"x{p}_{c}", name=f"xt{p}_{c}") for c in range(CT)]
                for p in range(2)
            ]

            def prefetch_x(gj):
                p = gj % 2
                for c in range(CT):
                    nc.gpsimd.dma_start(
                        out=xts[p][c][:, :],
                        in_=xT[c * 128 : (c + 1) * 128, gj * 512 : (gj + 1) * 512],
                    )

            def rope_apply(pc, gj, dst_of_h):
                """pc = [E(512)|O(512)] bf16 staging; write rotated per-head
                [d,512] into dst_of_h[h] (cols 0:512 of qhat, or the j-block
                cols of khat)."""
                E, O = pc[:, 0:512], pc[:, 512:1024]
                ce = cos_sb[:, gj * 512 : (gj + 1) * 512]
                se = sin_sb[:, gj * 512 : (gj + 1) * 512]
                t1 = rtpool.tile([128, 512], bf16, tag="rt")
                t2 = rtpool.tile([128, 512], bf16, tag="rt")
                nc.vector.tensor_mul(t1[:, :], E, ce)
                nc.vector.tensor_mul(t2[:, :], O, se)
                for h in range(HPC):
                    hb = 64 * h
                    dst, c0 = dst_of_h[h]
                    nc.vector.tensor_sub(
                        dst[0:64, c0 : c0 + 512], t1[hb : hb + 64, :], t2[hb : hb + 64, :]
                    )
                t3 = rtpool.tile([128, 512], bf16, tag="rt")
                t4 = rtpool.tile([128, 512], bf16, tag="rt")
                nc.vector.tensor_mul(t3[:, :], E, se)
                nc.vector.tensor_mul(t4[:, :], O, ce)
                for h in range(HPC):
                    hb = 64 * h
                    dst, c0 = dst_of_h[h]
                    nc.vector.tensor_add(
                        dst[64:128, c0 : c0 + 512], t3[hb : hb + 64, :], t4[hb : hb + 64, :]
                    )

            def qkv_units(gj):
                """PE filler units for block gj's projections. Q/K run as four
                single-PSUM-bank passes (QE, QO, KE, KO); each drains to half
                of a bf16 staging tile, rope fires after the O half."""
                b, j = divmod(gj, NB)
                p = gj % 2
                units = []

                def qk_passes(part0, dst_of_h):
                    # part0 = 0 for Q (QE,QO), 2 for K (KE,KO)
                    pc = pcpool.tile([128, 1024], bf16, tag="pc", name="pc")
                    ps_e = [
                        pf.tile([128, 512], f32, tag="f", name=f"ps_qk{e}")
                        for e in range(2)
                    ]

                    def unit(e, c0):
                        pt = part0 + e
                        for c in range(c0, c0 + 4):
                            wsl = wqk_sb[:, c * 512 + pt * 128 : c * 512 + (pt + 1) * 128]
                            nc.tensor.matmul(
                                ps_e[e][:, :],
                                wsl,
                                xts[p][c][:, :],
                                start=(c == 0),
                                stop=(c == CT - 1),
                            )
                        if c0 + 4 == CT:
                            nc.scalar.copy(pc[:, e * 512 : (e + 1) * 512], ps_e[e][:, :])
                            if e == 1:
                                rope_apply(pc, gj, dst_of_h)
                    return [
                        lambda e=e, c0=c0: unit(e, c0)
                        for e in range(2)
                        for c0 in range(0, CT, 4)
                    ]

                units += qk_passes(0, [(qhat[p][h], 0) for h in range(HPC)])
                units += qk_passes(2, [(khat[b][h], j * 512) for h in range(HPC)])

                def v_unit(tl):
                    ps = pf.tile([128, 256], f32, tag="f", name="ps_v")
                    for c in range(CT):
                        nc.tensor.matmul(
                            ps[:, :],
                            xts[p][c][:, tl * 128 : (tl + 1) * 128],
                            wv_sb[:, c * 256 : (c + 1) * 256],
                            start=(c == 0),
                            stop=(c == CT - 1),
                        )
                    base = (j * 4 + tl) * VW
                    for h in range(HPC):
                        nc.scalar.copy(
                            v_sb[b][:, base + h * 130 : base + h * 130 + 128],
                            ps[:, h * 128 : (h + 1) * 128],
                        )
                units += [lambda tl=tl: v_unit(tl) for tl in range(4)]
                return units

            def wo_units(gj):
                """PE filler units for block gj's output projection."""
                b, j = divmod(gj, NB)
                units = []

                def tt_unit(tt, ob2):
                    # two 512-wide o-chains for token tile tt
                    if ob2 == 0:
                        yo = yopool.tile([128, 2048], bf16, tag="yo", name=f"yo_{gj}_{tt}")
                        tt_unit.yo = yo
                    yo = tt_unit.yo
                    tsl = slice((j * 4 + tt) * 128, (j * 4 + tt) * 128 + 128)
                    for ob in (2 * ob2, 2 * ob2 + 1):
                        ps = pf.tile([128, 512], f32, tag="f", name="o_ps")
                        for h in range(HPC):
                            nc.tensor.matmul(
                                ps[:, :],
                                yT[b][h][:, tsl],
                                wo_sb[:, h * C + ob * 512 : h * C + (ob + 1) * 512],
                                start=(h == 0),
                                stop=(h == HPC - 1),
                            )
                        dsl = yo[:, ob * 512 : (ob + 1) * 512]
                        if ob % 2 == 0:
                            nc.scalar.copy(dsl, ps[:, :])
                        else:
                            nc.vector.tensor_copy(dsl, ps[:, :])
                    if ob2 == 1:
                        n0 = b * T + (j * 4 + tt) * 128
                        nc.sync.dma_start(out=out_p[n0 : n0 + 128, :], in_=yo[:, :])
                for tt in range(4):
                    for ob2 in range(2):
                        units.append(lambda tt=tt, ob2=ob2: tt_unit(tt, ob2))
                return units

            def attention(gj, filler):
                """Emit attention for block gj, pulling filler units between
                dependency-stalled PE instructions."""
                b, j = divmod(gj, NB)
                p = gj % 2

                def pull(k=1):
                    for _ in range(k):
                        if filler:
                            filler.popleft()()

                for h in range(HPC):
                    exs = []
                    for i in range(4 * j + 4):
                        sc = psc.tile([128, 512], f32, tag="sc", name="sc")
                        nc.tensor.matmul(
                            sc[:, :],
                            khat[b][h][:, i * 128 : (i + 1) * 128],
                            qhat[p][h][:, :],
                            start=True,
                            stop=True,
                        )
                        ex = expool.tile([128, 512], bf16, tag="ex")
                        nc.scalar.activation(
                            ex[:, :], sc[:, :],
                            mybir.ActivationFunctionType.Exp, scale=SCALE,
                        )
                        pdiag = i - 4 * j
                        if pdiag >= 0:
                            nc.vector.tensor_mul(
                                ex[:, :], ex[:, :],
                                mask_sb[:, pdiag * 512 : (pdiag + 1) * 512],
                            )
                        exs.append(ex)
                        pull()
                    for tau in range(4):
                        g = 4 * j + tau
                        y_ps = py.tile([128, 129], f32, tag="y", name="y_ps")
                        for i in range(g + 1):
                            nc.tensor.matmul(
                                y_ps[:, :],
                                exs[i][:, tau * 128 : (tau + 1) * 128],
                                v_sb[b][:, i * VW + h * 130 : i * VW + h * 130 + 129],
                                start=(i == 0),
                                stop=(i == g),
                            )
                        r = rpool.tile([128, 1], f32, tag="r")
                        nc.vector.reciprocal(r[:, :], y_ps[:, 128:129])
                        y_sb = ypool.tile([128, 128], bf16, tag="y")
                        nc.vector.tensor_scalar_mul(y_sb[:, :], y_ps[:, 0:128], r[:, 0:1])
                        yt_ps = psc.tile([128, 128], bf16, tag="sc", name="yt_ps")
                        nc.tensor.transpose(yt_ps[:, :], y_sb[:, :], ident[:, :])
                        gcol = (j * 4 + tau) * 128
                        nc.vector.tensor_copy(yT[b][h][:, gcol : gcol + 128], yt_ps[:, :])
                        pull()

            # ---------------- schedule ----------------
            # qdeq: next block's projections — must finish within the window
            # (flushed at window end). wdeq: WO backlog — drained lazily as
            # filler so the tail attention still has PE work, flushed at end.
            from collections import deque

            prefetch_x(0)
            prefetch_x(1)
            for u in qkv_units(0):
                u()
            qdeq = deque()
            wdeq = deque()

            class F:
                def __init__(self, q, w):
                    self.q, self.w = q, w

                def __bool__(self):
                    return bool(self.q) or bool(self.w)

                def popleft(self):
                    return self.q.popleft() if self.q else self.w.popleft()

            filler = F(qdeq, wdeq)
            for gj in range(NBLK):
                if gj + 2 < NBLK:
                    prefetch_x(gj + 2)
                if gj + 1 < NBLK:
                    qdeq.extend(qkv_units(gj + 1))
                attention(gj, filler)
                while qdeq:
                    qdeq.popleft()()
                wdeq.extend(wo_units(gj))
            while wdeq:
                wdeq.popleft()()
    nc.finalize()
    return nc


def _prep_inputs(x, w_qkv, w_o, rope_cos, rope_sin):
    import ml_dtypes

    bf = ml_dtypes.bfloat16
    xTh = np.ascontiguousarray(x.reshape(N, C).T).astype(bf)
    cosT = np.ascontiguousarray(rope_cos.T)  # [64, T]
    sinT = np.ascontiguousarray(rope_sin.T)
    cos2 = np.tile(np.concatenate([cosT, cosT], 0), (1, B)).astype(bf)
    sin2 = np.tile(np.concatenate([sinT, sinT], 0), (1, B)).astype(bf)

    r = np.arange(128)[:, None]
    c = np.arange(512)[None, :]
    singles = [((c - r) >= 128 * p).astype(np.float32) for p in range(4)]
    mk = np.concatenate(singles, axis=1).astype(bf)

    ev = np.arange(0, D, 2)
    od = np.arange(1, D, 2)
    in_maps = []
    for m in range(NCORES):
        h0, h1 = 2 * m, 2 * m + 1
        # blocks QE|QO|KE|KO; within each, cols = [head0 dims | head1 dims]
        QE = np.concatenate([w_qkv[h0 * D + ev, :], w_qkv[h1 * D + ev, :]], 0).T
        QO = np.concatenate([w_qkv[h0 * D + od, :], w_qkv[h1 * D + od, :]], 0).T
        KE = np.concatenate([w_qkv[C + h0 * D + ev, :], w_qkv[C + h1 * D + ev, :]], 0).T
        KO = np.concatenate([w_qkv[C + h0 * D + od, :], w_qkv[C + h1 * D + od, :]], 0).T
        wqk_m = np.ascontiguousarray(np.concatenate([QE, QO, KE, KO], 1)).astype(bf)
        wv_m = np.ascontiguousarray(
            w_qkv[2 * C + 2 * m * D : 2 * C + (2 * m + 2) * D, :].T
        ).astype(bf)
        wo_m = np.ascontiguousarray(w_o[:, 2 * m * D : (2 * m + 2) * D].T).astype(bf)
        in_maps.append(
            {
                "xT": xTh,
                "w_qk": wqk_m,
                "w_v": wv_m,
                "w_o": wo_m,
                "cos2": cos2,
                "sin2": sin2,
                "masks": np.ascontiguousarray(mk),
            }
        )
    return in_maps


def kernel(x, w_qkv, w_o, rope_cos, rope_sin, _trace=False):
    global _COMPILED
    x = np.asarray(x, dtype=np.float32)
    w_qkv = np.asarray(w_qkv, dtype=np.float32)
    w_o = np.asarray(w_o, dtype=np.float32)
    rope_cos = np.asarray(rope_cos, dtype=np.float32)
    rope_sin = np.asarray(rope_sin, dtype=np.float32)

    from concourse.bass_utils import run_bass_kernel_spmd

    if _COMPILED is None:
        _COMPILED = _build()
    nc = _COMPILED
    in_maps = _prep_inputs(x, w_qkv, w_o, rope_cos, rope_sin)
    res = run_bass_kernel_spmd(
        nc, in_maps, core_ids=list(range(NCORES)), trace=_trace
    )
    out = np.zeros((N, C), dtype=np.float32)
    for m in range(NCORES):
        out += np.asarray(res.results[m]["out_p"], dtype=np.float32)
    kernel._last_results = res
    return out.reshape(B, T, C)
